# revision 8
# baseline (speedup 1.0000x reference)
"""LIF (leaky integrate-and-fire) spiking recurrence on 8 Trainium2 cores.

Full input x: [T*bs, C, H, W] = [256, 128, 32, 32] f32 with T=8, bs=32.
Recurrence over T only, elementwise elsewhere:
    u_t = TAU * u_{t-1} * (1 - (u_{t-1} > VTH)) + x_t ;  o_t = (u_t > VTH)

Sharding: fully data-parallel over batch (bs=32 -> 4 per core), no collectives.

Since the op is elementwise outside of T, each core views its [4,128,32,32]
per-timestep slab as a flat [128 partitions, 4096] tile (16 KiB contiguous
HBM run per partition -> large DMA descriptors). Each timestep is split into
CH chunks so compute and output stores start as early as possible; the two
chunk chains interleave on DVE and hide cross-engine stalls.

Per step and chunk:
  DVE : u = (p * TAU) + x_t            (scalar_tensor_tensor; t=0: u=x_0)
  ACT : s = sign(VTH - u); o = relu(-s) = (u > VTH)   (exact: u-VTH exact near VTH)
  DVE : p = (u <= VTH) * u             (skipped at t=T-1)
TAU=0.5 is a power of two and the masks are 0/1, so everything except the
add is exact -> bitwise identical to the f32 reference.
"""

import numpy as np

import concourse.tile as tile
from concourse import bacc, mybir
from concourse.bass_utils import run_bass_kernel_spmd

T = 8
BS = 32
C = 128
HW = 32 * 32
NCORES = 8
BSH = BS // NCORES          # 4 batch elements per core
P = 128                     # SBUF partitions
FREE = BSH * C * HW // P    # 4096 f32 per partition per timestep
CH = 2                      # chunks per timestep
CHF = FREE // CH            # 2048
VTH = 1.0
TAU = 0.5
F32 = mybir.dt.float32

_nc_cache = None


def _build():
    nc = bacc.Bacc("TRN2", target_bir_lowering=False, debug=False, num_devices=NCORES)
    x_d = nc.dram_tensor("x", [T, P, FREE], F32, kind="ExternalInput").ap()
    o_d = nc.dram_tensor("o", [T, P, FREE], F32, kind="ExternalOutput").ap()

    chunks = [(t, c) for t in range(T) for c in range(CH)]
    PRE = 12  # chunks of x prefetched ahead (12 MiB of the 16 MiB input)

    with tile.TileContext(nc) as tc:
        with (
            tc.tile_pool(name="xp", bufs=PRE) as xp,
            tc.tile_pool(name="up", bufs=3) as up,
            tc.tile_pool(name="pp", bufs=2) as pp,
            tc.tile_pool(name="sp", bufs=2) as sp,
            tc.tile_pool(name="op", bufs=4) as op,
        ):
            def load(i):
                t, c = chunks[i]
                xt = xp.tile([P, CHF], F32, name="xt", tag="xt")
                nc.sync.dma_start(
                    out=xt[:], in_=x_d[t][:, c * CHF:(c + 1) * CHF]
                )
                return xt

            # Issue the whole prefetch window up front so the SP ring runs
            # all input loads before it parks on the first store's wait; the
            # input is then fully resident by ~40us and the endgame fabric
            # belongs to the stores.
            xtiles = [load(i) for i in range(PRE)]

            p = [None] * CH
            for i, (t, c) in enumerate(chunks):
                xt = xtiles[i]
                if t == 0:
                    u = xt  # u_0 = x_0 since u starts at 0
                else:
                    u = up.tile([P, CHF], F32)
                    nc.vector.scalar_tensor_tensor(
                        u[:], p[c][:], TAU, xt[:],
                        op0=mybir.AluOpType.mult, op1=mybir.AluOpType.add,
                    )
                # s = sign(VTH - u); o = relu(-s) = (u > VTH). Signs are
                # flipped via the scale immediate because only 0.0/1.0
                # have pre-registered const APs for the bias operand.
                s = sp.tile([P, CHF], F32)
                nc.scalar.activation(
                    s[:], u[:], mybir.ActivationFunctionType.Sign,
                    bias=VTH, scale=-1.0,
                )
                o = op.tile([P, CHF], F32)
                nc.scalar.activation(
                    o[:], s[:], mybir.ActivationFunctionType.Relu, scale=-1.0
                )
                if t < T - 1:
                    p[c] = pp.tile([P, CHF], F32, name="p", tag="p")
                    nc.vector.scalar_tensor_tensor(
                        p[c][:], u[:], VTH, u[:],
                        op0=mybir.AluOpType.is_le, op1=mybir.AluOpType.mult,
                    )
                if i + PRE < len(chunks):
                    xtiles.append(load(i + PRE))
                # Stores ride the SP ring behind the loads (HWDGE). The tail
                # stores are split finer so the final transfer is short.
                if t == T - 1:
                    h = CHF // 2
                    nc.sync.dma_start(
                        out=o_d[t][:, c * CHF:c * CHF + h], in_=o[:, :h]
                    )
                    nc.sync.dma_start(
                        out=o_d[t][:, c * CHF + h:(c + 1) * CHF], in_=o[:, h:]
                    )
                else:
                    nc.sync.dma_start(
                        out=o_d[t][:, c * CHF:(c + 1) * CHF], in_=o[:]
                    )

    nc.compile()
    return nc


def _get_nc():
    global _nc_cache
    if _nc_cache is None:
        _nc_cache = _build()
    return _nc_cache


def _run(x: np.ndarray, **spmd_kwargs):
    nc = _get_nc()
    xr = np.ascontiguousarray(np.asarray(x, dtype=np.float32)).reshape(T, BS, C, HW)
    in_maps = [
        {"x": np.ascontiguousarray(xr[:, k * BSH:(k + 1) * BSH]).reshape(T, P, FREE)}
        for k in range(NCORES)
    ]
    res = run_bass_kernel_spmd(nc, in_maps, core_ids=list(range(NCORES)), **spmd_kwargs)
    out = np.empty((T, BS, C, HW), dtype=np.float32)
    for k in range(NCORES):
        out[:, k * BSH:(k + 1) * BSH] = res.results[k]["o"].reshape(T, BSH, C, HW)
    return out.reshape(T * BS, C, 32, 32), res


def kernel(x: np.ndarray) -> np.ndarray:
    out, _ = _run(x)
    return out
